# revision 4
# baseline (speedup 1.0000x reference)
"""Trainium2 Bass kernel for nn_DenseBlock_MHSA (dense_cnn).

Data-parallel over batch across 8 NeuronCores (512 samples/core).
Layout: channel-major activations [ch, tokens] on chip, tokens = 512*9 = 4608/core.
Main GEMMs run as float32r (full PE rate at N>=256, ~1.5e-4 rel err); the small
per-head attention matmuls run bf16. BN batch stats for the two attention
outputs are all-reduced across cores; x's BN stats are computed host-side
(x is a kernel input, so its stats are layout/prep work).
"""

import numpy as np
import ml_dtypes

import concourse.bass as bass
import concourse.mybir as mybir
import concourse.tile as tile
from concourse import bacc
from concourse.bass_utils import run_bass_kernel_spmd

F32 = mybir.dt.float32
F32R = mybir.dt.float32r
BF16 = mybir.dt.bfloat16
AF = mybir.ActivationFunctionType
ALU = mybir.AluOpType

N_CORES = 8
B, C, F, HW = 4096, 512, 512, 9
HEADS, D = 4, 128
BC = B // N_CORES            # samples per core
T = BC * HW                  # tokens per core
EPS = 1e-5
NTOT = float(B * HW)         # global token count for BN stats

CH_S = 56                    # samples per chunk
NT_F = CH_S * HW             # 504 tokens per full chunk
GS = 14 * HW                 # 126 tokens per attention group
CHUNKS = [(c * CH_S, CH_S) for c in range(BC // CH_S)]
if BC % CH_S:
    CHUNKS.append(((BC // CH_S) * CH_S, BC % CH_S))

# vecs column map
VEC = {n: i for i, n in enumerate(
    ["s1", "t1", "s2x", "t2x", "s3x", "t3x", "b1", "qb", "kb", "b2", "b3",
     "g2a", "b2a", "g3a", "b3a", "g3b", "b3b", "vb"])}
NV = len(VEC)

_cache = {}


def _build():
    nc = bacc.Bacc("TRN2", target_bir_lowering=False, debug=False,
                   num_devices=N_CORES)

    dram = {}

    def din(name, shape, dt):
        dram[name] = nc.dram_tensor(name, shape, dt, kind="ExternalInput").ap()
        return dram[name]

    x_d = din("x_cm", [C, T], F32)
    w1_d = din("w1T", [C, F], F32R)
    wq_d = din("wqT", [F, F], F32R)
    wk_d = din("wkT", [F, F], F32R)
    wv_d = din("wvT", [F, F], F32R)
    w2x_d = din("w2Tx", [C, F], F32R)
    w2a_d = din("w2Ta", [F, F], BF16)
    w3x_d = din("w3Tx", [C, F], F32R)
    w3a_d = din("w3Ta", [F, F], BF16)
    w3b_d = din("w3Tb", [F, F], BF16)
    vecs_d = din("vecs", [C, NV], F32)
    posrep_d = din("posrep", [D, HEADS * GS], BF16)
    mlhs_d = din("mask_lhs", [15, GS], BF16)
    mrhs_d = din("mask_rhs", [15, GS], BF16)
    ones_d = din("ones126", [GS, 128], BF16)
    out_d = nc.dram_tensor("out_cm", [F, T], F32, kind="ExternalOutput").ap()

    with tile.TileContext(nc) as tc:
        from contextlib import ExitStack
        es = ExitStack()
        cpool = es.enter_context(tc.tile_pool(name="consts", bufs=1))
        apool = es.enter_context(tc.tile_pool(name="attres", bufs=1))
        dpool = es.enter_context(tc.tile_pool(name="dram", bufs=1, space="DRAM"))
        work = es.enter_context(tc.tile_pool(name="work", bufs=2))
        ps = es.enter_context(tc.tile_pool(name="ps", bufs=2, space="PSUM"))

        def ldconst(name, dr, shape, dt):
            t_ = cpool.tile(shape, dt, name=name, tag=name)
            nc.sync.dma_start(t_[:], dr[:])
            return t_

        posrep = ldconst("posrep", posrep_d, [D, HEADS * GS], BF16)
        mlhs = ldconst("mlhs", mlhs_d, [15, GS], BF16)
        mrhs = ldconst("mrhs", mrhs_d, [15, GS], BF16)
        ones126 = ldconst("ones126", ones_d, [GS, 128], BF16)
        vec = [ldconst(f"vec{k}", vecs_d[128 * k:128 * (k + 1), :], [128, NV], F32)
               for k in range(4)]

        def vslice(k, name):
            i = VEC[name]
            return vec[k][:, i:i + 1]

        # stat-derived per-channel vectors (computed after all-reduces)
        sv = {}
        for nm in ["s2a", "t2a", "s3a", "t3a", "s3b", "t3b"]:
            sv[nm] = [cpool.tile([128, 1], F32, name=f"{nm}_{k}", tag=f"{nm}_{k}")
                      for k in range(4)]

        # persistent attention outputs (bf16, channel-major)
        o3_att = [apool.tile([128, T], BF16, name=f"o3att{k}", tag=f"o3att{k}")
                  for k in range(4)]
        o7_att = [apool.tile([128, T], BF16, name=f"o7att{k}", tag=f"o7att{k}")
                  for k in range(4)]

        def wtiles(pool, name, dr, dt):
            ts = []
            for k in range(4):
                t_ = pool.tile([128, F], dt, name=f"{name}{k}", tag=f"{name}{k}")
                nc.sync.dma_start(t_[:], dr[128 * k:128 * (k + 1), :])
                ts.append(t_)
            return ts

        def groups_of(ns):
            """(tok_off_in_chunk, gs_tokens) attention groups for ns samples"""
            out = []
            s = 0
            while s < ns:
                g = min(14, ns - s)
                out.append((s * HW, g * HW))
                s += g
            return out

        def conv_gemm(wlist, srcs, nt, och):
            """accumulate sum_k w[k][:,och].T @ srcs[k] into a fresh psum"""
            p = ps.tile([128, NT_F], F32, name="mmps", tag="mmps", bufs=3)
            n = len(srcs)
            for k in range(n):
                nc.tensor.matmul(p[:, :nt], wlist[k][:, 128 * och:128 * (och + 1)],
                                 srcs[k][:, :nt], start=(k == 0), stop=(k == n - 1))
            return p

        def mhsa(o3t, w_q, w_k, w_v, dest, t0, nt, ns):
            """o3t: 4 input ch-tiles [128, nt] f32r; dest: 4 persistent bf16
            tiles, written at [:, t0:t0+nt]."""
            grps = groups_of(ns)
            qs, ks_ = [], []
            for h in range(HEADS):
                p = conv_gemm(w_q, o3t, nt, h)
                qh = work.tile([128, NT_F], BF16, name="qh", tag=f"qh{h}")
                nc.vector.tensor_scalar(qh[:, :nt], p[:, :nt], vslice(h, "qb"),
                                        None, ALU.add)
                qs.append(qh)
                p = conv_gemm(w_k, o3t, nt, h)
                kh = work.tile([128, NT_F], BF16, name="kh", tag=f"kh{h}")
                nc.scalar.activation(kh[:, :nt], p[:, :nt], AF.Identity,
                                     bias=vslice(h, "kb"))
                ks_.append(kh)
            vts = []
            for gi, (g0, gs) in enumerate(grps):
                p = ps.tile([GS, F], F32, name="mmps_v", tag="mmps", bufs=3)
                for k in range(4):
                    nc.tensor.matmul(p[:gs, :], o3t[k][:, g0:g0 + gs],
                                     w_v[k][:], start=(k == 0), stop=(k == 3))
                vt = work.tile([GS, F], BF16, name="vt", tag=f"vt{gi}")
                nc.vector.tensor_copy(vt[:gs, :], p[:gs, :])
                vts.append(vt)
            for h in range(HEADS):
                L = ps.tile([GS, NT_F], F32, name="Lps", tag="L", bufs=2)
                for gi, (g0, gs) in enumerate(grps):
                    sl = slice(g0, g0 + gs)
                    nc.tensor.matmul(L[:gs, sl], ks_[h][:, sl], qs[h][:, sl],
                                     start=True, stop=False)
                    nc.tensor.matmul(L[:gs, sl], qs[h][:, sl],
                                     posrep[:, GS * h:GS * h + gs],
                                     start=False, stop=False)
                    nc.tensor.matmul(L[:gs, sl], mlhs[:, :gs], mrhs[:, :gs],
                                     start=False, stop=True)
                gsmax = grps[0][1]
                E = work.tile([GS, NT_F], BF16, name="E", tag="E")
                nc.scalar.activation(E[:gsmax, :nt], L[:gsmax, :nt], AF.Exp)
                Db = ps.tile([128, NT_F], F32, name="Dbps", tag="Db", bufs=1)
                nc.tensor.matmul(Db[:, :nt], ones126[:gsmax, :], E[:gsmax, :nt],
                                 start=True, stop=True)
                rcp = work.tile([128, NT_F], F32, name="rcp", tag="rcp")
                nc.vector.reciprocal_approx_fast(rcp[:, :nt], Db[:, :nt])
                num = ps.tile([128, NT_F], F32, name="numps", tag="num", bufs=2)
                for gi, (g0, gs) in enumerate(grps):
                    sl = slice(g0, g0 + gs)
                    nc.tensor.matmul(num[:, sl], vts[gi][:gs, 128 * h:128 * (h + 1)],
                                     E[:gs, sl], start=True, stop=True)
                nc.vector.tensor_tensor(dest[h][:, t0 + 0:t0 + nt], num[:, :nt],
                                        rcp[:, :nt], ALU.mult)

        def stats_and_vectors(att, svs, svt, svs2, svt2, gnames, ar_tag):
            """bn stats of 4 bf16 tiles [128, T] -> allreduce -> scale/shift."""
            arp = work.tile([128, 8], F32, name=f"arp{ar_tag}", tag="arp", bufs=1)
            for k in range(4):
                st = work.tile([128, 9 * 6], F32, name="bnst", tag="bnst")
                for i in range(9):
                    nc.vector.bn_stats(st[:, 6 * i:6 * (i + 1)],
                                       att[k][:, 512 * i:512 * (i + 1)])
                ag = work.tile([128, 2], F32, name="bnag", tag="bnag")
                nc.vector.bn_aggr(ag[:], st[:])
                nc.vector.tensor_scalar(arp[:, k:k + 1], ag[:, 0:1], float(T),
                                        None, ALU.mult)
                sq = work.tile([128, 1], F32, name="sq", tag="sq")
                nc.vector.tensor_tensor(sq[:], ag[:, 0:1], ag[:, 0:1], ALU.mult)
                nc.vector.tensor_tensor(sq[:], sq[:], ag[:, 1:2], ALU.add)
                nc.vector.tensor_scalar(arp[:, 4 + k:5 + k], sq[:], float(T),
                                        None, ALU.mult)
            ar_in = dpool.tile([128, 8], F32, name=f"ar_in{ar_tag}",
                               tag=f"ar_in{ar_tag}")
            ar_out = dpool.tile([128, 8], F32, name=f"ar_out{ar_tag}",
                                tag=f"ar_out{ar_tag}", addr_space="Shared")
            nc.gpsimd.dma_start(ar_in[:], arp[:])
            nc.gpsimd.collective_compute(
                "AllReduce", ALU.add,
                replica_groups=[list(range(N_CORES))],
                ins=[ar_in.opt()], outs=[ar_out.opt()])
            arr = work.tile([128, 8], F32, name=f"arr{ar_tag}", tag="arr", bufs=1)
            nc.gpsimd.dma_start(arr[:], ar_out[:])
            for k in range(4):
                mean = work.tile([128, 1], F32, name="mean", tag="mean")
                nc.vector.tensor_scalar(mean[:], arr[:, k:k + 1], 1.0 / NTOT,
                                        None, ALU.mult)
                var = work.tile([128, 1], F32, name="var", tag="var")
                nc.vector.tensor_scalar(var[:], arr[:, 4 + k:5 + k], 1.0 / NTOT,
                                        None, ALU.mult)
                msq = work.tile([128, 1], F32, name="msq", tag="msq")
                nc.vector.tensor_tensor(msq[:], mean[:], mean[:], ALU.mult)
                nc.vector.tensor_tensor(var[:], var[:], msq[:], ALU.subtract)
                u = work.tile([128, 1], F32, name="u", tag="u")
                nc.vector.tensor_scalar(u[:], var[:], EPS, None, ALU.add)
                ru = work.tile([128, 1], F32, name="ru", tag="ru")
                nc.vector.reciprocal(ru[:], u[:])
                y0 = work.tile([128, 1], F32, name="y0", tag="y0")
                nc.scalar.activation(y0[:], ru[:], AF.Sqrt)
                # newton: y1 = y0 * (1.5 - 0.5*u*y0^2)  (rsqrt refine)
                y2 = work.tile([128, 1], F32, name="y2", tag="y2")
                nc.vector.tensor_tensor(y2[:], y0[:], y0[:], ALU.mult)
                nc.vector.tensor_tensor(y2[:], y2[:], u[:], ALU.mult)
                nc.vector.tensor_scalar(y2[:], y2[:], -0.5, 1.5, ALU.mult, ALU.add)
                nc.vector.tensor_tensor(y2[:], y2[:], y0[:], ALU.mult)
                # mprime = mean + vb
                nc.vector.tensor_tensor(mean[:], mean[:], vslice(k, "vb"), ALU.add)
                for (sname, tname, gn, bn) in ((svs, svt, gnames[0], gnames[1]),
                                               (svs2, svt2, gnames[2], gnames[3])):
                    if sname is None:
                        continue
                    nc.vector.tensor_tensor(sname[k][:], vslice(k, gn), y2[:],
                                            ALU.mult)
                    tm = work.tile([128, 1], F32, name="tm", tag="tm")
                    nc.vector.tensor_tensor(tm[:], mean[:], sname[k][:], ALU.mult)
                    nc.vector.tensor_tensor(tname[k][:], vslice(k, bn), tm[:],
                                            ALU.subtract)

        def load_x_chunk(t0, nt, sname, tname):
            """DMA x chunk + ACT(relu, s, t) -> f32r tiles"""
            xn = []
            for k in range(4):
                xc = work.tile([128, NT_F], F32, name="xc", tag=f"xc{k}", bufs=1)
                nc.sync.dma_start(xc[:, :nt], x_d[128 * k:128 * (k + 1), t0:t0 + nt])
                xnk = work.tile([128, NT_F], F32R, name="xn", tag=f"xn{k}")
                nc.scalar.activation(xnk[:, :nt], xc[:, :nt], AF.Relu,
                                     bias=vslice(k, tname), scale=vslice(k, sname))
                xn.append(xnk)
            return xn

        # ---------------- phase 1 ----------------
        with tc.tile_pool(name="wqkv", bufs=1) as wqkv_pool:
            w_q = wtiles(wqkv_pool, "wq", wq_d, F32R)
            w_k = wtiles(wqkv_pool, "wk", wk_d, F32R)
            w_v = wtiles(wqkv_pool, "wv", wv_d, F32R)

            with tc.tile_pool(name="w1p", bufs=1) as w1pool:
                w_1 = wtiles(w1pool, "w1", w1_d, F32R)
                for (s0, ns) in CHUNKS:
                    t0, nt = s0 * HW, ns * HW
                    xn = load_x_chunk(t0, nt, "s1", "t1")
                    o3 = []
                    for o in range(4):
                        p = conv_gemm(w_1, xn, nt, o)
                        o3k = work.tile([128, NT_F], F32R, name="o3", tag=f"o3{o}")
                        nc.vector.tensor_scalar(o3k[:, :nt], p[:, :nt],
                                                vslice(o, "b1"), None, ALU.add)
                        o3.append(o3k)
                    mhsa(o3, w_q, w_k, w_v, o3_att, t0, nt, ns)

            stats_and_vectors(o3_att, sv["s2a"], sv["t2a"], sv["s3a"], sv["t3a"],
                              ("g2a", "b2a", "g3a", "b3a"), "1")

            # ---------------- phase 2 ----------------
            with tc.tile_pool(name="w2p", bufs=1) as w2pool:
                w_2x = wtiles(w2pool, "w2x", w2x_d, F32R)
                w_2a = wtiles(w2pool, "w2a", w2a_d, BF16)
                for (s0, ns) in CHUNKS:
                    t0, nt = s0 * HW, ns * HW
                    xn = load_x_chunk(t0, nt, "s2x", "t2x")
                    o3a = []
                    for k in range(4):
                        a = work.tile([128, NT_F], BF16, name="o3a", tag=f"o3a{k}")
                        nc.scalar.activation(a[:, :nt], o3_att[k][:, t0:t0 + nt],
                                             AF.Relu, bias=sv["t2a"][k][:],
                                             scale=sv["s2a"][k][:])
                        o3a.append(a)
                    o7 = []
                    for o in range(4):
                        p = ps.tile([128, NT_F], F32, name="mmps2", tag="mmps",
                                    bufs=3)
                        for k in range(4):
                            nc.tensor.matmul(p[:, :nt],
                                             w_2x[k][:, 128 * o:128 * (o + 1)],
                                             xn[k][:, :nt], start=(k == 0),
                                             stop=False)
                        for k in range(4):
                            nc.tensor.matmul(p[:, :nt],
                                             w_2a[k][:, 128 * o:128 * (o + 1)],
                                             o3a[k][:, :nt], start=False,
                                             stop=(k == 3))
                        o7k = work.tile([128, NT_F], F32R, name="o7", tag=f"o3{o}")
                        nc.vector.tensor_scalar(o7k[:, :nt], p[:, :nt],
                                                vslice(o, "b2"), None, ALU.add)
                        o7.append(o7k)
                    mhsa(o7, w_q, w_k, w_v, o7_att, t0, nt, ns)

        stats_and_vectors(o7_att, sv["s3b"], sv["t3b"], None, None,
                          ("g3b", "b3b", None, None), "2")

        # ---------------- phase 3 ----------------
        with tc.tile_pool(name="w3p", bufs=1) as w3pool:
            w_3x = wtiles(w3pool, "w3x", w3x_d, F32R)
            w_3a = wtiles(w3pool, "w3a", w3a_d, BF16)
            w_3b = wtiles(w3pool, "w3b", w3b_d, BF16)
            for (s0, ns) in CHUNKS:
                t0, nt = s0 * HW, ns * HW
                xn = load_x_chunk(t0, nt, "s3x", "t3x")
                o3a = []
                o7a = []
                for k in range(4):
                    a = work.tile([128, NT_F], BF16, name="o3a3", tag=f"o3a{k}")
                    nc.scalar.activation(a[:, :nt], o3_att[k][:, t0:t0 + nt],
                                         AF.Relu, bias=sv["t3a"][k][:],
                                         scale=sv["s3a"][k][:])
                    o3a.append(a)
                    b = work.tile([128, NT_F], BF16, name="o7a3", tag=f"o7a{k}")
                    nc.scalar.activation(b[:, :nt], o7_att[k][:, t0:t0 + nt],
                                         AF.Relu, bias=sv["t3b"][k][:],
                                         scale=sv["s3b"][k][:])
                    o7a.append(b)
                for o in range(4):
                    p = ps.tile([128, NT_F], F32, name="mmps3", tag="mmps", bufs=3)
                    for k in range(4):
                        nc.tensor.matmul(p[:, :nt],
                                         w_3x[k][:, 128 * o:128 * (o + 1)],
                                         xn[k][:, :nt], start=(k == 0), stop=False)
                    for k in range(4):
                        nc.tensor.matmul(p[:, :nt],
                                         w_3a[k][:, 128 * o:128 * (o + 1)],
                                         o3a[k][:, :nt], start=False, stop=False)
                    for k in range(4):
                        nc.tensor.matmul(p[:, :nt],
                                         w_3b[k][:, 128 * o:128 * (o + 1)],
                                         o7a[k][:, :nt], start=False, stop=(k == 3))
                    ot = work.tile([128, NT_F], F32, name="ot", tag=f"o3{o}")
                    nc.vector.tensor_scalar(ot[:, :nt], p[:, :nt],
                                            vslice(o, "b3"), None, ALU.add)
                    nc.sync.dma_start(out_d[128 * o:128 * (o + 1), t0:t0 + nt],
                                      ot[:, :nt])
        es.close()

    nc.compile()
    return nc


def _host_prep(inputs):
    g = {k: np.asarray(v, np.float32) for k, v in inputs.items()}
    x = g["x"]
    m = x.mean(axis=(0, 2, 3))
    v = x.var(axis=(0, 2, 3))
    rs = 1.0 / np.sqrt(v + EPS)

    def st(gam, bet):
        s = gam * rs
        return s, bet - m * s

    vec_cols = {}
    vec_cols["s1"], vec_cols["t1"] = st(g["bn1_g"], g["bn1_b"])
    vec_cols["s2x"], vec_cols["t2x"] = st(g["bn2_g"][:C], g["bn2_b"][:C])
    vec_cols["s3x"], vec_cols["t3x"] = st(g["bn3_g"][:C], g["bn3_b"][:C])
    vec_cols["b1"] = g["b1"]
    vec_cols["qb"] = g["q_b"]
    vec_cols["kb"] = g["k_b"]
    vec_cols["b2"] = g["b2"]
    vec_cols["b3"] = g["b3"]
    vec_cols["g2a"] = g["bn2_g"][C:]
    vec_cols["b2a"] = g["bn2_b"][C:]
    vec_cols["g3a"] = g["bn3_g"][C:2 * C]
    vec_cols["b3a"] = g["bn3_b"][C:2 * C]
    vec_cols["g3b"] = g["bn3_g"][2 * C:]
    vec_cols["b3b"] = g["bn3_b"][2 * C:]
    vec_cols["vb"] = g["v_b"]
    vecs = np.zeros((C, NV), np.float32)
    for n, i in VEC.items():
        vecs[:, i] = vec_cols[n]

    bf = ml_dtypes.bfloat16
    pos = (g["rel_h"] + g["rel_w"]).reshape(HEADS, D, HW)
    posrep = np.tile(pos, (1, 1, 14)).transpose(1, 0, 2).reshape(D, HEADS * GS)

    b_of = np.repeat(np.arange(14), HW)
    mask_lhs = np.zeros((15, GS), np.float32)
    mask_rhs = np.zeros((15, GS), np.float32)
    for p in range(14):
        mask_lhs[p] = 50.0 * (b_of == p)
        mask_rhs[p] = (b_of == p).astype(np.float32)
    mask_lhs[14] = 50.0
    mask_rhs[14] = -1.0

    shared = {
        "w1T": np.ascontiguousarray(g["w1"].T),
        "wqT": np.ascontiguousarray(g["q_w"].T),
        "wkT": np.ascontiguousarray(g["k_w"].T),
        "wvT": np.ascontiguousarray(g["v_w"].T),
        "w2Tx": np.ascontiguousarray(g["w2"].T[:C]),
        "w2Ta": np.ascontiguousarray(g["w2"].T[C:]).astype(bf),
        "w3Tx": np.ascontiguousarray(g["w3"].T[:C]),
        "w3Ta": np.ascontiguousarray(g["w3"].T[C:2 * C]).astype(bf),
        "w3Tb": np.ascontiguousarray(g["w3"].T[2 * C:]).astype(bf),
        "vecs": vecs,
        "posrep": posrep.astype(bf),
        "mask_lhs": mask_lhs.astype(bf),
        "mask_rhs": mask_rhs.astype(bf),
        "ones126": np.ones((GS, 128), np.float32).astype(bf),
    }
    x_cm = x.reshape(B, C, HW).transpose(1, 0, 2)  # [C, B, HW] view
    in_maps = []
    for c in range(N_CORES):
        xs = np.ascontiguousarray(
            x_cm[:, BC * c:BC * (c + 1), :]).reshape(C, T)
        in_maps.append(dict(shared, x_cm=xs))
    return in_maps


def kernel(**inputs):
    if "nc" not in _cache:
        _cache["nc"] = _build()
    nc = _cache["nc"]
    in_maps = _host_prep(inputs)
    res = run_bass_kernel_spmd(nc, in_maps, core_ids=list(range(N_CORES)))
    parts = [res.results[c]["out_cm"].reshape(F, BC, HW)
             for c in range(N_CORES)]
    full = np.concatenate(parts, axis=1)          # [F, B, HW]
    return np.ascontiguousarray(full.transpose(1, 0, 2)).reshape(B, F, 3, 3)


# revision 6
# speedup vs baseline: 1.2019x; 1.2019x over previous
"""Trainium2 Bass kernel for nn_DenseBlock_MHSA (dense_cnn).

Data-parallel over batch across 8 NeuronCores (512 samples/core).
Layout: channel-major activations [ch, tokens] on chip, tokens = 512*9 = 4608/core.
All GEMMs run fp16 (full PE rate, FWL weight loads, ~5e-4 input rounding);
accumulation is fp32 in PSUM. The softmax mask bakes in a -6 logit shift so
fp16 exp cannot overflow. BN batch stats for the two attention
outputs are all-reduced across cores; x's BN stats are computed host-side
(x is a kernel input, so its stats are layout/prep work).
"""

import numpy as np
import ml_dtypes

import concourse.bass as bass
import concourse.mybir as mybir
import concourse.tile as tile
from concourse import bacc
from concourse.bass_utils import run_bass_kernel_spmd

F32 = mybir.dt.float32
FP16 = mybir.dt.float16
AF = mybir.ActivationFunctionType
ALU = mybir.AluOpType

N_CORES = 8
B, C, F, HW = 4096, 512, 512, 9
HEADS, D = 4, 128
BC = B // N_CORES            # samples per core
T = BC * HW                  # tokens per core
EPS = 1e-5
NTOT = float(B * HW)         # global token count for BN stats

CH_S = 56                    # samples per chunk
NT_F = CH_S * HW             # 504 tokens per full chunk
GS = 14 * HW                 # 126 tokens per attention group
CHUNKS = [(c * CH_S, CH_S) for c in range(BC // CH_S)]
if BC % CH_S:
    CHUNKS.append(((BC // CH_S) * CH_S, BC % CH_S))

# vecs column map
VEC = {n: i for i, n in enumerate(
    ["s1", "t1", "s2x", "t2x", "s3x", "t3x", "b1", "qb", "kb", "b2", "b3",
     "g2a", "b2a", "g3a", "b3a", "g3b", "b3b", "vb"])}
NV = len(VEC)

_cache = {}


def _build():
    nc = bacc.Bacc("TRN2", target_bir_lowering=False, debug=False,
                   num_devices=N_CORES)

    dram = {}

    def din(name, shape, dt):
        dram[name] = nc.dram_tensor(name, shape, dt, kind="ExternalInput").ap()
        return dram[name]

    x_d = din("x_cm", [C, T], F32)
    w1_d = din("w1T", [C, F], FP16)
    wq_d = din("wqT", [F, F], FP16)
    wk_d = din("wkT", [F, F], FP16)
    wv_d = din("wvT", [F, F], FP16)
    w2x_d = din("w2Tx", [C, F], FP16)
    w2a_d = din("w2Ta", [F, F], FP16)
    w3x_d = din("w3Tx", [C, F], FP16)
    w3a_d = din("w3Ta", [F, F], FP16)
    w3b_d = din("w3Tb", [F, F], FP16)
    vecs_d = din("vecs", [C, NV], F32)
    posrep_d = din("posrep", [D, HEADS * GS], FP16)
    mlhs_d = din("mask_lhs", [15, GS], FP16)
    mrhs_d = din("mask_rhs", [15, NT_F], FP16)
    ones_d = din("ones126", [GS, 128], FP16)
    out_d = nc.dram_tensor("out_cm", [F, T], F32, kind="ExternalOutput").ap()

    with tile.TileContext(nc) as tc:
        from contextlib import ExitStack
        es = ExitStack()
        cpool = es.enter_context(tc.tile_pool(name="consts", bufs=1))
        apool = es.enter_context(tc.tile_pool(name="attres", bufs=1))
        dpool = es.enter_context(tc.tile_pool(name="dram", bufs=1, space="DRAM"))
        work = es.enter_context(tc.tile_pool(name="work", bufs=2))
        ps = es.enter_context(tc.tile_pool(name="ps", bufs=2, space="PSUM"))

        def ldconst(name, dr, shape, dt):
            t_ = cpool.tile(shape, dt, name=name, tag=name)
            nc.sync.dma_start(t_[:], dr[:])
            return t_

        posrep = ldconst("posrep", posrep_d, [D, HEADS * GS], FP16)
        mlhs = ldconst("mlhs", mlhs_d, [15, GS], FP16)
        mrhs_w = ldconst("mrhs_w", mrhs_d, [15, NT_F], FP16)
        ones126 = ldconst("ones126", ones_d, [GS, 128], FP16)
        vec = [ldconst(f"vec{k}", vecs_d[128 * k:128 * (k + 1), :], [128, NV], F32)
               for k in range(4)]

        def vslice(k, name):
            i = VEC[name]
            return vec[k][:, i:i + 1]

        # stat-derived per-channel vectors (computed after all-reduces)
        sv = {}
        for nm in ["s2a", "t2a", "s3a", "t3a", "s3b", "t3b"]:
            sv[nm] = [cpool.tile([128, 1], F32, name=f"{nm}_{k}", tag=f"{nm}_{k}")
                      for k in range(4)]

        # incremental bn_stats buffers: one 6-wide slot per chunk per k-tile
        NCH = len(CHUNKS)
        st3 = [cpool.tile([128, 6 * NCH], F32, name=f"st3_{k}", tag=f"st3_{k}")
               for k in range(4)]
        st7 = [cpool.tile([128, 6 * NCH], F32, name=f"st7_{k}", tag=f"st7_{k}")
               for k in range(4)]

        # persistent attention outputs (fp16, channel-major)
        o3_att = [apool.tile([128, T], FP16, name=f"o3att{k}", tag=f"o3att{k}")
                  for k in range(4)]
        o7_att = [apool.tile([128, T], FP16, name=f"o7att{k}", tag=f"o7att{k}")
                  for k in range(4)]

        def wtiles(pool, name, dr, dt):
            ts = []
            for k in range(4):
                t_ = pool.tile([128, F], dt, name=f"{name}{k}", tag=f"{name}{k}")
                nc.sync.dma_start(t_[:], dr[128 * k:128 * (k + 1), :])
                ts.append(t_)
            return ts

        def groups_of(ns):
            """(tok_off_in_chunk, gs_tokens) attention groups for ns samples"""
            out = []
            s = 0
            while s < ns:
                g = min(14, ns - s)
                out.append((s * HW, g * HW))
                s += g
            return out

        def conv_gemm(wlist, srcs, nt, och):
            """accumulate sum_k w[k][:,och].T @ srcs[k] into a fresh psum"""
            p = ps.tile([128, NT_F], F32, name="mmps", tag="mmps", bufs=3)
            n = len(srcs)
            for k in range(n):
                nc.tensor.matmul(p[:, :nt], wlist[k][:, 128 * och:128 * (och + 1)],
                                 srcs[k][:, :nt], start=(k == 0), stop=(k == n - 1))
            return p

        def mhsa(o3t, w_q, w_k, w_v, dest, t0, nt, ns, st, ci):
            """o3t: 4 input ch-tiles [128, nt] f32r; dest: 4 persistent bf16
            tiles, written at [:, t0:t0+nt]."""
            grps = groups_of(ns)
            qs, ks_ = [], []
            for h in range(HEADS):
                p = conv_gemm(w_q, o3t, nt, h)
                qh = work.tile([128, NT_F], FP16, name="qh", tag=f"qh{h}")
                nc.vector.tensor_scalar(qh[:, :nt], p[:, :nt], vslice(h, "qb"),
                                        None, ALU.add)
                qs.append(qh)
                p = conv_gemm(w_k, o3t, nt, h)
                kh = work.tile([128, NT_F], FP16, name="kh", tag=f"kh{h}")
                nc.scalar.activation(kh[:, :nt], p[:, :nt], AF.Identity,
                                     bias=vslice(h, "kb"))
                ks_.append(kh)
            vts = []
            for gi, (g0, gs) in enumerate(grps):
                p = ps.tile([GS, F], F32, name="mmps_v", tag="mmps", bufs=3)
                for k in range(4):
                    nc.tensor.matmul(p[:gs, :], o3t[k][:, g0:g0 + gs],
                                     w_v[k][:], start=(k == 0), stop=(k == 3))
                vt = work.tile([GS, F], FP16, name="vt", tag=f"vt{gi}")
                nc.vector.tensor_copy(vt[:gs, :], p[:gs, :])
                vts.append(vt)
            for h in range(HEADS):
                L = ps.tile([GS, NT_F], F32, name="Lps", tag="L", bufs=2)
                for gi, (g0, gs) in enumerate(grps):
                    sl = slice(g0, g0 + gs)
                    nc.tensor.matmul(L[:gs, sl], ks_[h][:, sl], qs[h][:, sl],
                                     start=(gi == 0), stop=False)
                    nc.tensor.matmul(L[:gs, sl], qs[h][:, sl],
                                     posrep[:, GS * h:GS * h + gs],
                                     start=False, stop=False)
                nc.tensor.matmul(L[:, :nt], mlhs[:], mrhs_w[:, :nt],
                                 start=False, stop=True)
                gsmax = grps[0][1]
                E = work.tile([GS, NT_F], FP16, name="E", tag="E")
                nc.scalar.activation(E[:gsmax, :nt], L[:gsmax, :nt], AF.Exp)
                Db = ps.tile([128, NT_F], F32, name="Dbps", tag="Db", bufs=1)
                nc.tensor.matmul(Db[:, :nt], ones126[:gsmax, :], E[:gsmax, :nt],
                                 start=True, stop=True)
                rcp = work.tile([128, NT_F], F32, name="rcp", tag="rcp")
                nc.vector.reciprocal_approx_fast(rcp[:, :nt], Db[:, :nt])
                num = ps.tile([128, NT_F], F32, name="numps", tag="num", bufs=2)
                for gi, (g0, gs) in enumerate(grps):
                    sl = slice(g0, g0 + gs)
                    nc.tensor.matmul(num[:, sl], vts[gi][:gs, 128 * h:128 * (h + 1)],
                                     E[:gs, sl], start=True, stop=True)
                nc.vector.tensor_tensor(dest[h][:, t0 + 0:t0 + nt], num[:, :nt],
                                        rcp[:, :nt], ALU.mult)
                nc.vector.bn_stats(st[h][:, 6 * ci:6 * ci + 6],
                                   dest[h][:, t0:t0 + nt])

        def stats_and_vectors(st, svs, svt, svs2, svt2, gnames, ar_tag):
            """aggregate per-chunk bn stats -> allreduce -> scale/shift vecs"""
            arp = work.tile([128, 8], F32, name=f"arp{ar_tag}", tag="arp", bufs=1)
            for k in range(4):
                ag = work.tile([128, 2], F32, name="bnag", tag="bnag")
                nc.vector.bn_aggr(ag[:], st[k][:])
                nc.vector.tensor_scalar(arp[:, k:k + 1], ag[:, 0:1], float(T),
                                        None, ALU.mult)
                sq = work.tile([128, 1], F32, name="sq", tag="sq")
                nc.vector.tensor_tensor(sq[:], ag[:, 0:1], ag[:, 0:1], ALU.mult)
                nc.vector.tensor_tensor(sq[:], sq[:], ag[:, 1:2], ALU.add)
                nc.vector.tensor_scalar(arp[:, 4 + k:5 + k], sq[:], float(T),
                                        None, ALU.mult)
            ar_in = dpool.tile([128, 8], F32, name=f"ar_in{ar_tag}",
                               tag=f"ar_in{ar_tag}")
            ar_out = dpool.tile([128, 8], F32, name=f"ar_out{ar_tag}",
                                tag=f"ar_out{ar_tag}", addr_space="Shared")
            nc.gpsimd.dma_start(ar_in[:], arp[:])
            nc.gpsimd.collective_compute(
                "AllReduce", ALU.add,
                replica_groups=[list(range(N_CORES))],
                ins=[ar_in.opt()], outs=[ar_out.opt()])
            arr = work.tile([128, 8], F32, name=f"arr{ar_tag}", tag="arr", bufs=1)
            nc.gpsimd.dma_start(arr[:], ar_out[:])
            for k in range(4):
                mean = work.tile([128, 1], F32, name="mean", tag="mean")
                nc.vector.tensor_scalar(mean[:], arr[:, k:k + 1], 1.0 / NTOT,
                                        None, ALU.mult)
                var = work.tile([128, 1], F32, name="var", tag="var")
                nc.vector.tensor_scalar(var[:], arr[:, 4 + k:5 + k], 1.0 / NTOT,
                                        None, ALU.mult)
                msq = work.tile([128, 1], F32, name="msq", tag="msq")
                nc.vector.tensor_tensor(msq[:], mean[:], mean[:], ALU.mult)
                nc.vector.tensor_tensor(var[:], var[:], msq[:], ALU.subtract)
                u = work.tile([128, 1], F32, name="u", tag="u")
                nc.vector.tensor_scalar(u[:], var[:], EPS, None, ALU.add)
                ru = work.tile([128, 1], F32, name="ru", tag="ru")
                nc.vector.reciprocal(ru[:], u[:])
                y0 = work.tile([128, 1], F32, name="y0", tag="y0")
                nc.scalar.activation(y0[:], ru[:], AF.Sqrt)
                # newton: y1 = y0 * (1.5 - 0.5*u*y0^2)  (rsqrt refine)
                y2 = work.tile([128, 1], F32, name="y2", tag="y2")
                nc.vector.tensor_tensor(y2[:], y0[:], y0[:], ALU.mult)
                nc.vector.tensor_tensor(y2[:], y2[:], u[:], ALU.mult)
                nc.vector.tensor_scalar(y2[:], y2[:], -0.5, 1.5, ALU.mult, ALU.add)
                nc.vector.tensor_tensor(y2[:], y2[:], y0[:], ALU.mult)
                # mprime = mean + vb
                nc.vector.tensor_tensor(mean[:], mean[:], vslice(k, "vb"), ALU.add)
                for (sname, tname, gn, bn) in ((svs, svt, gnames[0], gnames[1]),
                                               (svs2, svt2, gnames[2], gnames[3])):
                    if sname is None:
                        continue
                    nc.vector.tensor_tensor(sname[k][:], vslice(k, gn), y2[:],
                                            ALU.mult)
                    tm = work.tile([128, 1], F32, name="tm", tag="tm")
                    nc.vector.tensor_tensor(tm[:], mean[:], sname[k][:], ALU.mult)
                    nc.vector.tensor_tensor(tname[k][:], vslice(k, bn), tm[:],
                                            ALU.subtract)

        def load_x_chunk(t0, nt, sname, tname):
            """DMA x chunk + ACT(relu, s, t) -> f32r tiles"""
            xn = []
            for k in range(4):
                xc = work.tile([128, NT_F], F32, name="xc", tag=f"xc{k}", bufs=1)
                nc.sync.dma_start(xc[:, :nt], x_d[128 * k:128 * (k + 1), t0:t0 + nt])
                xnk = work.tile([128, NT_F], FP16, name="xn", tag=f"xn{k}")
                nc.scalar.activation(xnk[:, :nt], xc[:, :nt], AF.Relu,
                                     bias=vslice(k, tname), scale=vslice(k, sname))
                xn.append(xnk)
            return xn

        # ---------------- phase 1 ----------------
        with tc.tile_pool(name="wqkv", bufs=1) as wqkv_pool:
            w_q = wtiles(wqkv_pool, "wq", wq_d, FP16)
            w_k = wtiles(wqkv_pool, "wk", wk_d, FP16)
            w_v = wtiles(wqkv_pool, "wv", wv_d, FP16)

            with tc.tile_pool(name="w1p", bufs=1) as w1pool:
                w_1 = wtiles(w1pool, "w1", w1_d, FP16)
                for ci, (s0, ns) in enumerate(CHUNKS):
                    t0, nt = s0 * HW, ns * HW
                    xn = load_x_chunk(t0, nt, "s1", "t1")
                    o3 = []
                    for o in range(4):
                        p = conv_gemm(w_1, xn, nt, o)
                        o3k = work.tile([128, NT_F], FP16, name="o3", tag=f"o3{o}")
                        nc.vector.tensor_scalar(o3k[:, :nt], p[:, :nt],
                                                vslice(o, "b1"), None, ALU.add)
                        o3.append(o3k)
                    mhsa(o3, w_q, w_k, w_v, o3_att, t0, nt, ns, st3, ci)

            # preload sqrt table set while phase-1 tail still runs
            sqwarm = work.tile([128, 1], F32, name="sqwarm", tag="sqwarm", bufs=1)
            nc.scalar.activation(sqwarm[:], vslice(0, "s1"), AF.Sqrt)
            # prefetch phase-2 chunk-0 x (independent of the all-reduce)
            xn_pre2 = load_x_chunk(0, NT_F, "s2x", "t2x")
            stats_and_vectors(st3, sv["s2a"], sv["t2a"], sv["s3a"], sv["t3a"],
                              ("g2a", "b2a", "g3a", "b3a"), "1")

            # ---------------- phase 2 ----------------
            with tc.tile_pool(name="w2p", bufs=1) as w2pool:
                w_2x = wtiles(w2pool, "w2x", w2x_d, FP16)
                w_2a = wtiles(w2pool, "w2a", w2a_d, FP16)
                for ci, (s0, ns) in enumerate(CHUNKS):
                    t0, nt = s0 * HW, ns * HW
                    xn = xn_pre2 if ci == 0 else load_x_chunk(t0, nt, "s2x", "t2x")
                    o3a = []
                    for k in range(4):
                        a = work.tile([128, NT_F], FP16, name="o3a", tag=f"o3a{k}")
                        nc.scalar.activation(a[:, :nt], o3_att[k][:, t0:t0 + nt],
                                             AF.Relu, bias=sv["t2a"][k][:],
                                             scale=sv["s2a"][k][:])
                        o3a.append(a)
                    o7 = []
                    for o in range(4):
                        p = ps.tile([128, NT_F], F32, name="mmps2", tag="mmps",
                                    bufs=3)
                        for k in range(4):
                            nc.tensor.matmul(p[:, :nt],
                                             w_2x[k][:, 128 * o:128 * (o + 1)],
                                             xn[k][:, :nt], start=(k == 0),
                                             stop=False)
                        for k in range(4):
                            nc.tensor.matmul(p[:, :nt],
                                             w_2a[k][:, 128 * o:128 * (o + 1)],
                                             o3a[k][:, :nt], start=False,
                                             stop=(k == 3))
                        o7k = work.tile([128, NT_F], FP16, name="o7", tag=f"o3{o}")
                        nc.vector.tensor_scalar(o7k[:, :nt], p[:, :nt],
                                                vslice(o, "b2"), None, ALU.add)
                        o7.append(o7k)
                    mhsa(o7, w_q, w_k, w_v, o7_att, t0, nt, ns, st7, ci)

        sqwarm2 = work.tile([128, 1], F32, name="sqwarm2", tag="sqwarm", bufs=1)
        nc.scalar.activation(sqwarm2[:], vslice(0, "s1"), AF.Sqrt)
        xn_pre3 = load_x_chunk(0, NT_F, "s3x", "t3x")
        stats_and_vectors(st7, sv["s3b"], sv["t3b"], None, None,
                          ("g3b", "b3b", None, None), "2")

        # ---------------- phase 3 ----------------
        with tc.tile_pool(name="w3p", bufs=1) as w3pool:
            w_3x = wtiles(w3pool, "w3x", w3x_d, FP16)
            w_3a = wtiles(w3pool, "w3a", w3a_d, FP16)
            w_3b = wtiles(w3pool, "w3b", w3b_d, FP16)
            for ci, (s0, ns) in enumerate(CHUNKS):
                t0, nt = s0 * HW, ns * HW
                xn = xn_pre3 if ci == 0 else load_x_chunk(t0, nt, "s3x", "t3x")
                o3a = []
                o7a = []
                for k in range(4):
                    a = work.tile([128, NT_F], FP16, name="o3a3", tag=f"o3a{k}")
                    nc.scalar.activation(a[:, :nt], o3_att[k][:, t0:t0 + nt],
                                         AF.Relu, bias=sv["t3a"][k][:],
                                         scale=sv["s3a"][k][:])
                    o3a.append(a)
                    b = work.tile([128, NT_F], FP16, name="o7a3", tag=f"o7a{k}")
                    nc.scalar.activation(b[:, :nt], o7_att[k][:, t0:t0 + nt],
                                         AF.Relu, bias=sv["t3b"][k][:],
                                         scale=sv["s3b"][k][:])
                    o7a.append(b)
                for o in range(4):
                    p = ps.tile([128, NT_F], F32, name="mmps3", tag="mmps", bufs=3)
                    for k in range(4):
                        nc.tensor.matmul(p[:, :nt],
                                         w_3x[k][:, 128 * o:128 * (o + 1)],
                                         xn[k][:, :nt], start=(k == 0), stop=False)
                    for k in range(4):
                        nc.tensor.matmul(p[:, :nt],
                                         w_3a[k][:, 128 * o:128 * (o + 1)],
                                         o3a[k][:, :nt], start=False, stop=False)
                    for k in range(4):
                        nc.tensor.matmul(p[:, :nt],
                                         w_3b[k][:, 128 * o:128 * (o + 1)],
                                         o7a[k][:, :nt], start=False, stop=(k == 3))
                    ot = work.tile([128, NT_F], F32, name="ot", tag=f"o3{o}")
                    nc.vector.tensor_scalar(ot[:, :nt], p[:, :nt],
                                            vslice(o, "b3"), None, ALU.add)
                    nc.sync.dma_start(out_d[128 * o:128 * (o + 1), t0:t0 + nt],
                                      ot[:, :nt])
        es.close()

    nc.compile()
    return nc


def _host_prep(inputs):
    g = {k: np.asarray(v, np.float32) for k, v in inputs.items()}
    x = g["x"]
    m = x.mean(axis=(0, 2, 3))
    v = x.var(axis=(0, 2, 3))
    rs = 1.0 / np.sqrt(v + EPS)

    def st(gam, bet):
        s = gam * rs
        return s, bet - m * s

    vec_cols = {}
    vec_cols["s1"], vec_cols["t1"] = st(g["bn1_g"], g["bn1_b"])
    vec_cols["s2x"], vec_cols["t2x"] = st(g["bn2_g"][:C], g["bn2_b"][:C])
    vec_cols["s3x"], vec_cols["t3x"] = st(g["bn3_g"][:C], g["bn3_b"][:C])
    vec_cols["b1"] = g["b1"]
    vec_cols["qb"] = g["q_b"]
    vec_cols["kb"] = g["k_b"]
    vec_cols["b2"] = g["b2"]
    vec_cols["b3"] = g["b3"]
    vec_cols["g2a"] = g["bn2_g"][C:]
    vec_cols["b2a"] = g["bn2_b"][C:]
    vec_cols["g3a"] = g["bn3_g"][C:2 * C]
    vec_cols["b3a"] = g["bn3_b"][C:2 * C]
    vec_cols["g3b"] = g["bn3_g"][2 * C:]
    vec_cols["b3b"] = g["bn3_b"][2 * C:]
    vec_cols["vb"] = g["v_b"]
    vecs = np.zeros((C, NV), np.float32)
    for n, i in VEC.items():
        vecs[:, i] = vec_cols[n]

    bf = np.float16
    pos = (g["rel_h"] + g["rel_w"]).reshape(HEADS, D, HW)
    posrep = np.tile(pos, (1, 1, 14)).transpose(1, 0, 2).reshape(D, HEADS * GS)

    b_of = np.repeat(np.arange(14), HW)
    mask_lhs = np.zeros((15, GS), np.float32)
    mask_rhs = np.zeros((15, GS), np.float32)
    for p in range(14):
        mask_lhs[p] = 50.0 * (b_of == p)
        mask_rhs[p] = (b_of == p).astype(np.float32)
    mask_lhs[14] = 50.0
    mask_rhs[14] = -1.12

    shared = {
        "w1T": np.ascontiguousarray(g["w1"].T).astype(bf),
        "wqT": np.ascontiguousarray(g["q_w"].T).astype(bf),
        "wkT": np.ascontiguousarray(g["k_w"].T).astype(bf),
        "wvT": np.ascontiguousarray(g["v_w"].T).astype(bf),
        "w2Tx": np.ascontiguousarray(g["w2"].T[:C]).astype(bf),
        "w2Ta": np.ascontiguousarray(g["w2"].T[C:]).astype(bf),
        "w3Tx": np.ascontiguousarray(g["w3"].T[:C]).astype(bf),
        "w3Ta": np.ascontiguousarray(g["w3"].T[C:2 * C]).astype(bf),
        "w3Tb": np.ascontiguousarray(g["w3"].T[2 * C:]).astype(bf),
        "vecs": vecs,
        "posrep": posrep.astype(bf),
        "mask_lhs": mask_lhs.astype(bf),
        "mask_rhs": np.tile(mask_rhs, (1, 4)).astype(bf),
        "ones126": np.ones((GS, 128), np.float32).astype(bf),
    }
    x_cm = x.reshape(B, C, HW).transpose(1, 0, 2)  # [C, B, HW] view
    in_maps = []
    for c in range(N_CORES):
        xs = np.ascontiguousarray(
            x_cm[:, BC * c:BC * (c + 1), :]).reshape(C, T)
        in_maps.append(dict(shared, x_cm=xs))
    return in_maps


def kernel(**inputs):
    if "nc" not in _cache:
        _cache["nc"] = _build()
    nc = _cache["nc"]
    in_maps = _host_prep(inputs)
    res = run_bass_kernel_spmd(nc, in_maps, core_ids=list(range(N_CORES)))
    parts = [res.results[c]["out_cm"].reshape(F, BC, HW)
             for c in range(N_CORES)]
    full = np.concatenate(parts, axis=1)          # [F, B, HW]
    return np.ascontiguousarray(full.transpose(1, 0, 2)).reshape(B, F, 3, 3)


# revision 7
# speedup vs baseline: 1.2576x; 1.0463x over previous
"""Trainium2 Bass kernel for nn_DenseBlock_MHSA (dense_cnn).

Data-parallel over batch across 8 NeuronCores (512 samples/core).
Layout: channel-major activations [ch, tokens] on chip, tokens = 512*9 = 4608/core.
All GEMMs run fp16 (full PE rate, FWL weight loads, ~5e-4 input rounding);
accumulation is fp32 in PSUM. The softmax mask bakes in a -6 logit shift so
fp16 exp cannot overflow. BN batch stats for the two attention
outputs are all-reduced across cores; x's BN stats are computed host-side
(x is a kernel input, so its stats are layout/prep work).
"""

import numpy as np
import ml_dtypes

import concourse.bass as bass
import concourse.mybir as mybir
import concourse.tile as tile
from concourse import bacc
from concourse.bass_utils import run_bass_kernel_spmd

F32 = mybir.dt.float32
FP16 = mybir.dt.float16
AF = mybir.ActivationFunctionType
ALU = mybir.AluOpType

N_CORES = 8
B, C, F, HW = 4096, 512, 512, 9
HEADS, D = 4, 128
BC = B // N_CORES            # samples per core
T = BC * HW                  # tokens per core
EPS = 1e-5
NTOT = float(B * HW)         # global token count for BN stats

CH_S = 56                    # samples per chunk
NT_F = CH_S * HW             # 504 tokens per full chunk
GS = 14 * HW                 # 126 tokens per attention group
CHUNKS = [(c * CH_S, CH_S) for c in range(BC // CH_S)]
if BC % CH_S:
    CHUNKS.append(((BC // CH_S) * CH_S, BC % CH_S))

# vecs column map
VEC = {n: i for i, n in enumerate(
    ["s1", "t1", "s2x", "t2x", "s3x", "t3x", "b1", "qb", "kb", "b2", "b3",
     "g2a", "b2a", "g3a", "b3a", "g3b", "b3b", "vb"])}
NV = len(VEC)

_cache = {}


def _build():
    nc = bacc.Bacc("TRN2", target_bir_lowering=False, debug=False,
                   num_devices=N_CORES)

    dram = {}

    def din(name, shape, dt):
        dram[name] = nc.dram_tensor(name, shape, dt, kind="ExternalInput").ap()
        return dram[name]

    x_d = din("x_cm", [C, T], F32)
    w1_d = din("w1T", [C, F], FP16)
    wq_d = din("wqT", [F, F], FP16)
    wk_d = din("wkT", [F, F], FP16)
    wv_d = din("wvT", [F, F], FP16)
    w2x_d = din("w2Tx", [C, F], FP16)
    w2a_d = din("w2Ta", [F, F], FP16)
    w3x_d = din("w3Tx", [C, F], FP16)
    w3a_d = din("w3Ta", [F, F], FP16)
    w3b_d = din("w3Tb", [F, F], FP16)
    vecs_d = din("vecs", [C, NV], F32)
    posrep_d = din("posrep", [D, HEADS * GS], FP16)
    mlhs_d = din("mask_lhs", [15, GS], FP16)
    mrhs_d = din("mask_rhs", [15, NT_F], FP16)
    ones_d = din("ones126", [GS, 128], FP16)
    out_d = nc.dram_tensor("out_cm", [F, T], F32, kind="ExternalOutput").ap()

    with tile.TileContext(nc) as tc:
        from contextlib import ExitStack
        es = ExitStack()
        cpool = es.enter_context(tc.tile_pool(name="consts", bufs=1))
        apool = es.enter_context(tc.tile_pool(name="attres", bufs=1))
        dpool = es.enter_context(tc.tile_pool(name="dram", bufs=1, space="DRAM"))
        work = es.enter_context(tc.tile_pool(name="work", bufs=2))
        ps = es.enter_context(tc.tile_pool(name="ps", bufs=2, space="PSUM"))

        def ldconst(name, dr, shape, dt):
            t_ = cpool.tile(shape, dt, name=name, tag=name)
            nc.sync.dma_start(t_[:], dr[:])
            return t_

        posrep = ldconst("posrep", posrep_d, [D, HEADS * GS], FP16)
        mlhs = ldconst("mlhs", mlhs_d, [15, GS], FP16)
        mrhs_w = ldconst("mrhs_w", mrhs_d, [15, NT_F], FP16)
        ones126 = ldconst("ones126", ones_d, [GS, 128], FP16)
        vec = [ldconst(f"vec{k}", vecs_d[128 * k:128 * (k + 1), :], [128, NV], F32)
               for k in range(4)]

        def vslice(k, name):
            i = VEC[name]
            return vec[k][:, i:i + 1]

        # stat-derived per-channel vectors (computed after all-reduces)
        sv = {}
        for nm in ["s2a", "t2a", "s3a", "t3a", "s3b", "t3b"]:
            sv[nm] = [cpool.tile([128, 1], F32, name=f"{nm}_{k}", tag=f"{nm}_{k}")
                      for k in range(4)]

        # incremental bn_stats buffers: one 6-wide slot per chunk per k-tile
        NCH = len(CHUNKS)
        st3 = [cpool.tile([128, 6 * NCH], F32, name=f"st3_{k}", tag=f"st3_{k}")
               for k in range(4)]
        st7 = [cpool.tile([128, 6 * NCH], F32, name=f"st7_{k}", tag=f"st7_{k}")
               for k in range(4)]

        # persistent attention outputs (fp16, channel-major)
        o3_att = [apool.tile([128, T], FP16, name=f"o3att{k}", tag=f"o3att{k}")
                  for k in range(4)]
        o7_att = [apool.tile([128, T], FP16, name=f"o7att{k}", tag=f"o7att{k}")
                  for k in range(4)]

        def wtiles(pool, name, dr, dt):
            ts = []
            for k in range(4):
                t_ = pool.tile([128, F], dt, name=f"{name}{k}", tag=f"{name}{k}")
                nc.gpsimd.dma_start(t_[:], dr[128 * k:128 * (k + 1), :])
                ts.append(t_)
            return ts

        def groups_of(ns):
            """(tok_off_in_chunk, gs_tokens) attention groups for ns samples"""
            out = []
            s = 0
            while s < ns:
                g = min(14, ns - s)
                out.append((s * HW, g * HW))
                s += g
            return out

        def conv_gemm(wlist, srcs, nt, och):
            """accumulate sum_k w[k][:,och].T @ srcs[k] into a fresh psum"""
            p = ps.tile([128, NT_F], F32, name="mmps", tag="mmps", bufs=4)
            n = len(srcs)
            for k in range(n):
                nc.tensor.matmul(p[:, :nt], wlist[k][:, 128 * och:128 * (och + 1)],
                                 srcs[k][:, :nt], start=(k == 0), stop=(k == n - 1))
            return p

        def mhsa(o3t, w_q, w_k, w_v, dest, t0, nt, ns, st, ci):
            """o3t: 4 input ch-tiles [128, nt] f32r; dest: 4 persistent bf16
            tiles, written at [:, t0:t0+nt]."""
            grps = groups_of(ns)
            qs, ks_ = [], []
            for h in range(HEADS):
                p = conv_gemm(w_q, o3t, nt, h)
                qh = work.tile([128, NT_F], FP16, name="qh", tag=f"qh{h}")
                nc.vector.tensor_scalar(qh[:, :nt], p[:, :nt], vslice(h, "qb"),
                                        None, ALU.add)
                qs.append(qh)
                p = conv_gemm(w_k, o3t, nt, h)
                kh = work.tile([128, NT_F], FP16, name="kh", tag=f"kh{h}")
                nc.scalar.activation(kh[:, :nt], p[:, :nt], AF.Identity,
                                     bias=vslice(h, "kb"))
                ks_.append(kh)
            vts = []
            for gi, (g0, gs) in enumerate(grps):
                p = ps.tile([GS, F], F32, name="mmps_v", tag="mmps", bufs=4)
                for k in range(4):
                    nc.tensor.matmul(p[:gs, :], o3t[k][:, g0:g0 + gs],
                                     w_v[k][:], start=(k == 0), stop=(k == 3))
                vt = work.tile([GS, F], FP16, name="vt", tag=f"vt{gi}")
                nc.vector.tensor_copy(vt[:gs, :], p[:gs, :])
                vts.append(vt)
            for h in range(HEADS):
                L = ps.tile([GS, NT_F], F32, name="Lps", tag="L", bufs=1)
                for gi, (g0, gs) in enumerate(grps):
                    sl = slice(g0, g0 + gs)
                    nc.tensor.matmul(L[:gs, sl], ks_[h][:, sl], qs[h][:, sl],
                                     start=(gi == 0), stop=False)
                    nc.tensor.matmul(L[:gs, sl], qs[h][:, sl],
                                     posrep[:, GS * h:GS * h + gs],
                                     start=False, stop=False)
                nc.tensor.matmul(L[:, :nt], mlhs[:], mrhs_w[:, :nt],
                                 start=False, stop=True)
                gsmax = grps[0][1]
                E = work.tile([GS, NT_F], FP16, name="E", tag="E")
                nc.scalar.activation(E[:gsmax, :nt], L[:gsmax, :nt], AF.Exp)
                Db = ps.tile([128, NT_F], F32, name="Dbps", tag="Db", bufs=1)
                nc.tensor.matmul(Db[:, :nt], ones126[:gsmax, :], E[:gsmax, :nt],
                                 start=True, stop=True)
                rcp = work.tile([128, NT_F], F32, name="rcp", tag="rcp")
                nc.vector.reciprocal_approx_fast(rcp[:, :nt], Db[:, :nt])
                num = ps.tile([128, NT_F], F32, name="numps", tag="num", bufs=2)
                for gi, (g0, gs) in enumerate(grps):
                    sl = slice(g0, g0 + gs)
                    nc.tensor.matmul(num[:, sl], vts[gi][:gs, 128 * h:128 * (h + 1)],
                                     E[:gs, sl], start=True, stop=True)
                nc.vector.tensor_tensor(dest[h][:, t0 + 0:t0 + nt], num[:, :nt],
                                        rcp[:, :nt], ALU.mult)
                nc.vector.bn_stats(st[h][:, 6 * ci:6 * ci + 6],
                                   dest[h][:, t0:t0 + nt])

        def stats_and_vectors(st, svs, svt, svs2, svt2, gnames, ar_tag):
            """aggregate per-chunk bn stats -> allreduce -> scale/shift vecs"""
            arp = work.tile([128, 8], F32, name=f"arp{ar_tag}", tag="arp", bufs=1)
            for k in range(4):
                ag = work.tile([128, 2], F32, name="bnag", tag="bnag")
                nc.vector.bn_aggr(ag[:], st[k][:])
                nc.vector.tensor_scalar(arp[:, k:k + 1], ag[:, 0:1], float(T),
                                        None, ALU.mult)
                sq = work.tile([128, 1], F32, name="sq", tag="sq")
                nc.vector.tensor_tensor(sq[:], ag[:, 0:1], ag[:, 0:1], ALU.mult)
                nc.vector.tensor_tensor(sq[:], sq[:], ag[:, 1:2], ALU.add)
                nc.vector.tensor_scalar(arp[:, 4 + k:5 + k], sq[:], float(T),
                                        None, ALU.mult)
            ar_in = dpool.tile([128, 8], F32, name=f"ar_in{ar_tag}",
                               tag=f"ar_in{ar_tag}")
            ar_out = dpool.tile([128, 8], F32, name=f"ar_out{ar_tag}",
                                tag=f"ar_out{ar_tag}", addr_space="Shared")
            nc.gpsimd.dma_start(ar_in[:], arp[:])
            nc.gpsimd.collective_compute(
                "AllReduce", ALU.add,
                replica_groups=[list(range(N_CORES))],
                ins=[ar_in.opt()], outs=[ar_out.opt()])
            arr = work.tile([128, 8], F32, name=f"arr{ar_tag}", tag="arr", bufs=1)
            nc.gpsimd.dma_start(arr[:], ar_out[:])
            for k in range(4):
                mean = work.tile([128, 1], F32, name="mean", tag="mean")
                nc.vector.tensor_scalar(mean[:], arr[:, k:k + 1], 1.0 / NTOT,
                                        None, ALU.mult)
                var = work.tile([128, 1], F32, name="var", tag="var")
                nc.vector.tensor_scalar(var[:], arr[:, 4 + k:5 + k], 1.0 / NTOT,
                                        None, ALU.mult)
                msq = work.tile([128, 1], F32, name="msq", tag="msq")
                nc.vector.tensor_tensor(msq[:], mean[:], mean[:], ALU.mult)
                nc.vector.tensor_tensor(var[:], var[:], msq[:], ALU.subtract)
                u = work.tile([128, 1], F32, name="u", tag="u")
                nc.vector.tensor_scalar(u[:], var[:], EPS, None, ALU.add)
                ru = work.tile([128, 1], F32, name="ru", tag="ru")
                nc.vector.reciprocal(ru[:], u[:])
                y0 = work.tile([128, 1], F32, name="y0", tag="y0")
                nc.scalar.activation(y0[:], ru[:], AF.Sqrt)
                # newton: y1 = y0 * (1.5 - 0.5*u*y0^2)  (rsqrt refine)
                y2 = work.tile([128, 1], F32, name="y2", tag="y2")
                nc.vector.tensor_tensor(y2[:], y0[:], y0[:], ALU.mult)
                nc.vector.tensor_tensor(y2[:], y2[:], u[:], ALU.mult)
                nc.vector.tensor_scalar(y2[:], y2[:], -0.5, 1.5, ALU.mult, ALU.add)
                nc.vector.tensor_tensor(y2[:], y2[:], y0[:], ALU.mult)
                # mprime = mean + vb
                nc.vector.tensor_tensor(mean[:], mean[:], vslice(k, "vb"), ALU.add)
                for (sname, tname, gn, bn) in ((svs, svt, gnames[0], gnames[1]),
                                               (svs2, svt2, gnames[2], gnames[3])):
                    if sname is None:
                        continue
                    nc.vector.tensor_tensor(sname[k][:], vslice(k, gn), y2[:],
                                            ALU.mult)
                    tm = work.tile([128, 1], F32, name="tm", tag="tm")
                    nc.vector.tensor_tensor(tm[:], mean[:], sname[k][:], ALU.mult)
                    nc.vector.tensor_tensor(tname[k][:], vslice(k, bn), tm[:],
                                            ALU.subtract)

        def load_x_chunk(t0, nt, sname, tname):
            """DMA x chunk + ACT(relu, s, t) -> f32r tiles"""
            xn = []
            for k in range(4):
                xc = work.tile([128, NT_F], F32, name="xc", tag=f"xc{k}", bufs=1)
                nc.sync.dma_start(xc[:, :nt], x_d[128 * k:128 * (k + 1), t0:t0 + nt])
                xnk = work.tile([128, NT_F], FP16, name="xn", tag=f"xn{k}")
                nc.scalar.activation(xnk[:, :nt], xc[:, :nt], AF.Relu,
                                     bias=vslice(k, tname), scale=vslice(k, sname))
                xn.append(xnk)
            return xn

        # ---------------- phase 1 ----------------
        with tc.tile_pool(name="wqkv", bufs=1) as wqkv_pool:
            w_q = wtiles(wqkv_pool, "wq", wq_d, FP16)
            w_k = wtiles(wqkv_pool, "wk", wk_d, FP16)
            w_v = wtiles(wqkv_pool, "wv", wv_d, FP16)

            with tc.tile_pool(name="w1p", bufs=1) as w1pool:
                w_1 = wtiles(w1pool, "w1", w1_d, FP16)
                for ci, (s0, ns) in enumerate(CHUNKS):
                    t0, nt = s0 * HW, ns * HW
                    xn = load_x_chunk(t0, nt, "s1", "t1")
                    o3 = []
                    for o in range(4):
                        p = conv_gemm(w_1, xn, nt, o)
                        o3k = work.tile([128, NT_F], FP16, name="o3", tag=f"o3{o}")
                        nc.vector.tensor_scalar(o3k[:, :nt], p[:, :nt],
                                                vslice(o, "b1"), None, ALU.add)
                        o3.append(o3k)
                    mhsa(o3, w_q, w_k, w_v, o3_att, t0, nt, ns, st3, ci)

            # preload sqrt table set while phase-1 tail still runs
            sqwarm = work.tile([128, 1], F32, name="sqwarm", tag="sqwarm", bufs=1)
            nc.scalar.activation(sqwarm[:], vslice(0, "s1"), AF.Sqrt)
            # prefetch phase-2 chunk-0 x (independent of the all-reduce)
            xn_pre2 = load_x_chunk(0, NT_F, "s2x", "t2x")
            stats_and_vectors(st3, sv["s2a"], sv["t2a"], sv["s3a"], sv["t3a"],
                              ("g2a", "b2a", "g3a", "b3a"), "1")

            # ---------------- phase 2 ----------------
            with tc.tile_pool(name="w2p", bufs=1) as w2pool:
                w_2x = wtiles(w2pool, "w2x", w2x_d, FP16)
                w_2a = wtiles(w2pool, "w2a", w2a_d, FP16)
                for ci, (s0, ns) in enumerate(CHUNKS):
                    t0, nt = s0 * HW, ns * HW
                    xn = xn_pre2 if ci == 0 else load_x_chunk(t0, nt, "s2x", "t2x")
                    o3a = []
                    for k in range(4):
                        a = work.tile([128, NT_F], FP16, name="o3a", tag=f"o3a{k}")
                        nc.scalar.activation(a[:, :nt], o3_att[k][:, t0:t0 + nt],
                                             AF.Relu, bias=sv["t2a"][k][:],
                                             scale=sv["s2a"][k][:])
                        o3a.append(a)
                    o7 = []
                    pps = []
                    for o in range(4):
                        p = ps.tile([128, NT_F], F32, name="mmps2", tag="mmps",
                                    bufs=4)
                        for k in range(4):
                            nc.tensor.matmul(p[:, :nt],
                                             w_2x[k][:, 128 * o:128 * (o + 1)],
                                             xn[k][:, :nt], start=(k == 0),
                                             stop=False)
                        pps.append(p)
                    for o in range(4):
                        p = pps[o]
                        for k in range(4):
                            nc.tensor.matmul(p[:, :nt],
                                             w_2a[k][:, 128 * o:128 * (o + 1)],
                                             o3a[k][:, :nt], start=False,
                                             stop=(k == 3))
                        o7k = work.tile([128, NT_F], FP16, name="o7", tag=f"o3{o}")
                        nc.vector.tensor_scalar(o7k[:, :nt], p[:, :nt],
                                                vslice(o, "b2"), None, ALU.add)
                        o7.append(o7k)
                    mhsa(o7, w_q, w_k, w_v, o7_att, t0, nt, ns, st7, ci)

        sqwarm2 = work.tile([128, 1], F32, name="sqwarm2", tag="sqwarm", bufs=1)
        nc.scalar.activation(sqwarm2[:], vslice(0, "s1"), AF.Sqrt)
        xn_pre3 = load_x_chunk(0, NT_F, "s3x", "t3x")
        stats_and_vectors(st7, sv["s3b"], sv["t3b"], None, None,
                          ("g3b", "b3b", None, None), "2")

        # ---------------- phase 3 ----------------
        with tc.tile_pool(name="w3p", bufs=1) as w3pool:
            w_3x = wtiles(w3pool, "w3x", w3x_d, FP16)
            w_3a = wtiles(w3pool, "w3a", w3a_d, FP16)
            w_3b = wtiles(w3pool, "w3b", w3b_d, FP16)
            for ci, (s0, ns) in enumerate(CHUNKS):
                t0, nt = s0 * HW, ns * HW
                xn = xn_pre3 if ci == 0 else load_x_chunk(t0, nt, "s3x", "t3x")
                o3a = []
                o7a = []
                for k in range(4):
                    a = work.tile([128, NT_F], FP16, name="o3a3", tag=f"o3a{k}")
                    nc.scalar.activation(a[:, :nt], o3_att[k][:, t0:t0 + nt],
                                         AF.Relu, bias=sv["t3a"][k][:],
                                         scale=sv["s3a"][k][:])
                    o3a.append(a)
                    b = work.tile([128, NT_F], FP16, name="o7a3", tag=f"o7a{k}")
                    nc.scalar.activation(b[:, :nt], o7_att[k][:, t0:t0 + nt],
                                         AF.Relu, bias=sv["t3b"][k][:],
                                         scale=sv["s3b"][k][:])
                    o7a.append(b)
                pps = []
                for o in range(4):
                    p = ps.tile([128, NT_F], F32, name="mmps3", tag="mmps", bufs=4)
                    for k in range(4):
                        nc.tensor.matmul(p[:, :nt],
                                         w_3x[k][:, 128 * o:128 * (o + 1)],
                                         xn[k][:, :nt], start=(k == 0), stop=False)
                    for k in range(4):
                        nc.tensor.matmul(p[:, :nt],
                                         w_3a[k][:, 128 * o:128 * (o + 1)],
                                         o3a[k][:, :nt], start=False, stop=False)
                    pps.append(p)
                for o in range(4):
                    p = pps[o]
                    for k in range(4):
                        nc.tensor.matmul(p[:, :nt],
                                         w_3b[k][:, 128 * o:128 * (o + 1)],
                                         o7a[k][:, :nt], start=False, stop=(k == 3))
                    ot = work.tile([128, NT_F], F32, name="ot", tag=f"o3{o}")
                    nc.vector.tensor_scalar(ot[:, :nt], p[:, :nt],
                                            vslice(o, "b3"), None, ALU.add)
                    nc.sync.dma_start(out_d[128 * o:128 * (o + 1), t0:t0 + nt],
                                      ot[:, :nt])
        es.close()

    nc.compile()
    return nc


def _host_prep(inputs):
    g = {k: np.asarray(v, np.float32) for k, v in inputs.items()}
    x = g["x"]
    m = x.mean(axis=(0, 2, 3))
    v = x.var(axis=(0, 2, 3))
    rs = 1.0 / np.sqrt(v + EPS)

    def st(gam, bet):
        s = gam * rs
        return s, bet - m * s

    vec_cols = {}
    vec_cols["s1"], vec_cols["t1"] = st(g["bn1_g"], g["bn1_b"])
    vec_cols["s2x"], vec_cols["t2x"] = st(g["bn2_g"][:C], g["bn2_b"][:C])
    vec_cols["s3x"], vec_cols["t3x"] = st(g["bn3_g"][:C], g["bn3_b"][:C])
    vec_cols["b1"] = g["b1"]
    vec_cols["qb"] = g["q_b"]
    vec_cols["kb"] = g["k_b"]
    vec_cols["b2"] = g["b2"]
    vec_cols["b3"] = g["b3"]
    vec_cols["g2a"] = g["bn2_g"][C:]
    vec_cols["b2a"] = g["bn2_b"][C:]
    vec_cols["g3a"] = g["bn3_g"][C:2 * C]
    vec_cols["b3a"] = g["bn3_b"][C:2 * C]
    vec_cols["g3b"] = g["bn3_g"][2 * C:]
    vec_cols["b3b"] = g["bn3_b"][2 * C:]
    vec_cols["vb"] = g["v_b"]
    vecs = np.zeros((C, NV), np.float32)
    for n, i in VEC.items():
        vecs[:, i] = vec_cols[n]

    bf = np.float16
    pos = (g["rel_h"] + g["rel_w"]).reshape(HEADS, D, HW)
    posrep = np.tile(pos, (1, 1, 14)).transpose(1, 0, 2).reshape(D, HEADS * GS)

    b_of = np.repeat(np.arange(14), HW)
    mask_lhs = np.zeros((15, GS), np.float32)
    mask_rhs = np.zeros((15, GS), np.float32)
    for p in range(14):
        mask_lhs[p] = 50.0 * (b_of == p)
        mask_rhs[p] = (b_of == p).astype(np.float32)
    mask_lhs[14] = 50.0
    mask_rhs[14] = -1.12

    shared = {
        "w1T": np.ascontiguousarray(g["w1"].T).astype(bf),
        "wqT": np.ascontiguousarray(g["q_w"].T).astype(bf),
        "wkT": np.ascontiguousarray(g["k_w"].T).astype(bf),
        "wvT": np.ascontiguousarray(g["v_w"].T).astype(bf),
        "w2Tx": np.ascontiguousarray(g["w2"].T[:C]).astype(bf),
        "w2Ta": np.ascontiguousarray(g["w2"].T[C:]).astype(bf),
        "w3Tx": np.ascontiguousarray(g["w3"].T[:C]).astype(bf),
        "w3Ta": np.ascontiguousarray(g["w3"].T[C:2 * C]).astype(bf),
        "w3Tb": np.ascontiguousarray(g["w3"].T[2 * C:]).astype(bf),
        "vecs": vecs,
        "posrep": posrep.astype(bf),
        "mask_lhs": mask_lhs.astype(bf),
        "mask_rhs": np.tile(mask_rhs, (1, 4)).astype(bf),
        "ones126": np.ones((GS, 128), np.float32).astype(bf),
    }
    x_cm = x.reshape(B, C, HW).transpose(1, 0, 2)  # [C, B, HW] view
    in_maps = []
    for c in range(N_CORES):
        xs = np.ascontiguousarray(
            x_cm[:, BC * c:BC * (c + 1), :]).reshape(C, T)
        in_maps.append(dict(shared, x_cm=xs))
    return in_maps


def kernel(**inputs):
    if "nc" not in _cache:
        _cache["nc"] = _build()
    nc = _cache["nc"]
    in_maps = _host_prep(inputs)
    res = run_bass_kernel_spmd(nc, in_maps, core_ids=list(range(N_CORES)))
    parts = [res.results[c]["out_cm"].reshape(F, BC, HW)
             for c in range(N_CORES)]
    full = np.concatenate(parts, axis=1)          # [F, B, HW]
    return np.ascontiguousarray(full.transpose(1, 0, 2)).reshape(B, F, 3, 3)
